# revision 1
# baseline (speedup 1.0000x reference)
"""EMA (exponential moving average) linear-recurrence kernel for TRN2, 8 cores.

y_t = w*x_t + (1-w)*y_{t-1}, inputs [B=16, T=8192, C=256] f32.

Strategy: pure data-parallel over batch (2 batches/core, no communication).
Per core, channels live on SBUF partitions (2 groups of 128) and time runs
along the free dimension, where the DVE tensor_tensor_scan instruction
computes the recurrence natively. DRAM layout is [T, C], so tiles are
transposed on-chip with the tensor engine (PE) in 128x128 blocks.

z-space trick: z_t = a*z_{t-1} + x_t with z_init = y0/w gives y_t = w*z_t,
so no pre-scale pass is needed; the w scale is folded into the transpose-back
matmul by using diag(w) instead of the identity. Channels with w ~ 0 are
fixed up on the host (y_t = y0 exactly).
"""

import sys

sys.path.insert(0, "/opt/trn_rl_repo")

import numpy as np

B, T, C = 16, 8192, 256
N_CORES = 8
B_LOC = B // N_CORES          # 2 batches per core
P = 128                       # SBUF partitions
G = C // P                    # 2 channel groups
TB = 2048                     # timesteps per DMA block (2 MB per transfer)
NBLK = T // TB                # 4 blocks per batch
CHUNK = 512                   # timesteps per scan chunk (= 1 PSUM bank of f32)
NCHUNK = TB // CHUNK          # 4 chunks per block
SUB = CHUNK // P              # 4 PE 128x128 sub-tiles per chunk
K = TB // P                   # 16 sub-tiles per block

_compiled = None


def _build():
    import concourse.tile as tile
    from concourse import bacc, mybir
    from concourse.mybir import AluOpType

    nc = bacc.Bacc("TRN2", target_bir_lowering=False, debug=False,
                   num_devices=N_CORES)
    f32 = mybir.dt.float32

    x_ap = nc.dram_tensor("x", [B_LOC, T, C], f32, kind="ExternalInput").ap()
    abc_ap = nc.dram_tensor("abc", [P, G * CHUNK], f32, kind="ExternalInput").ap()
    wdiag_ap = nc.dram_tensor("wdiag", [P, G * P], f32, kind="ExternalInput").ap()
    ident_ap = nc.dram_tensor("ident", [P, P], f32, kind="ExternalInput").ap()
    z0c_ap = nc.dram_tensor("z0c", [P, B_LOC * G], f32, kind="ExternalInput").ap()
    y_ap = nc.dram_tensor("y", [B_LOC, T, C], f32, kind="ExternalOutput").ap()

    with tile.TileContext(nc) as tc:
        with (
            tc.tile_pool(name="const", bufs=1) as cpool,
            tc.tile_pool(name="xin", bufs=2) as xpool,
            tc.tile_pool(name="z", bufs=6) as zpool,
            tc.tile_pool(name="yout", bufs=2) as ypool,
            tc.tile_pool(name="xt", bufs=4, space="PSUM") as xtpool,
            tc.tile_pool(name="yt", bufs=4, space="PSUM") as ytpool,
        ):
            abc_t = cpool.tile([P, G * CHUNK], f32)
            nc.sync.dma_start(abc_t[:], abc_ap[:])
            wdiag_t = cpool.tile([P, G * P], f32)
            nc.sync.dma_start(wdiag_t[:], wdiag_ap[:])
            ident_t = cpool.tile([P, P], f32)
            nc.sync.dma_start(ident_t[:], ident_ap[:])
            z0c_t = cpool.tile([P, B_LOC * G], f32)
            nc.sync.dma_start(z0c_t[:], z0c_ap[:])

            for b in range(B_LOC):
                zprev = [z0c_t[:, b * G + g:b * G + g + 1] for g in range(G)]
                for blk in range(NBLK):
                    t0 = blk * TB
                    xin = xpool.tile([P, K, C], f32, tag="xin")
                    src = x_ap[b, t0:t0 + TB, :].rearrange(
                        "(k p) c -> p k c", p=P)
                    nc.sync.dma_start(xin[:], src)

                    yout = ypool.tile([P, K, C], f32, tag="yout")
                    for q in range(NCHUNK):
                        for g in range(G):
                            xt = xtpool.tile([P, CHUNK], f32, tag="xt")
                            for s in range(SUB):
                                k = q * SUB + s
                                nc.tensor.transpose(
                                    xt[:, s * P:(s + 1) * P],
                                    xin[:, k, g * P:(g + 1) * P],
                                    ident_t[:],
                                )
                            z = zpool.tile([P, CHUNK], f32, tag="z")
                            nc.vector.tensor_tensor_scan(
                                z[:],
                                abc_t[:, g * CHUNK:(g + 1) * CHUNK],
                                xt[:],
                                initial=zprev[g],
                                op0=AluOpType.mult,
                                op1=AluOpType.add,
                            )
                            zprev[g] = z[:, CHUNK - 1:CHUNK]
                            yt = ytpool.tile([P, CHUNK], f32, tag="yt")
                            for s in range(SUB):
                                nc.tensor.matmul(
                                    yt[:, s * P:(s + 1) * P],
                                    z[:, s * P:(s + 1) * P],
                                    wdiag_t[:, g * P:(g + 1) * P],
                                    start=True,
                                    stop=True,
                                )
                            nc.scalar.copy(
                                yout[:, q * SUB:(q + 1) * SUB, g * P:(g + 1) * P],
                                yt[:].rearrange("p (s c) -> p s c", s=SUB),
                            )
                    dst = y_ap[b, t0:t0 + TB, :].rearrange(
                        "(k p) c -> p k c", p=P)
                    nc.sync.dma_start(dst, yout[:])

    nc.compile()
    return nc


def _get_compiled():
    global _compiled
    if _compiled is None:
        _compiled = _build()
    return _compiled


def kernel(inputs, initial_state, smooth):
    from concourse.bass_utils import run_bass_kernel_spmd

    inputs = np.ascontiguousarray(inputs, dtype=np.float32)
    initial_state = np.ascontiguousarray(initial_state, dtype=np.float32)
    smooth = np.ascontiguousarray(smooth, dtype=np.float32)

    w = np.clip(smooth, 0.0, 1.0)
    a = 1.0 - w
    mask = w < 1e-30
    w_safe = np.where(mask, 1.0, w)
    z0 = initial_state / w_safe                      # [B, C]
    z0 = np.where(mask[None, :], 0.0, z0)

    # a broadcast along time, per channel group: abc[p, g*CHUNK + j] = a[g*128+p]
    abc = np.empty((P, G * CHUNK), dtype=np.float32)
    for g in range(G):
        abc[:, g * CHUNK:(g + 1) * CHUNK] = a[g * P:(g + 1) * P][:, None]
    # diag(w) per group (masked channels scale to 0; fixed up below)
    wdiag = np.zeros((P, G * P), dtype=np.float32)
    for g in range(G):
        wg = np.where(mask[g * P:(g + 1) * P], 0.0, w[g * P:(g + 1) * P])
        wdiag[:, g * P:(g + 1) * P][np.arange(P), np.arange(P)] = wg
    ident = np.eye(P, dtype=np.float32)

    nc = _get_compiled()
    in_maps = []
    for c in range(N_CORES):
        bs = slice(c * B_LOC, (c + 1) * B_LOC)
        z0c = np.empty((P, B_LOC * G), dtype=np.float32)
        for b in range(B_LOC):
            for g in range(G):
                z0c[:, b * G + g] = z0[c * B_LOC + b, g * P:(g + 1) * P]
        in_maps.append({
            "x": inputs[bs],
            "abc": abc,
            "wdiag": wdiag,
            "ident": ident,
            "z0c": z0c,
        })

    res = run_bass_kernel_spmd(nc, in_maps, list(range(N_CORES)))
    out = np.concatenate([res.results[c]["y"] for c in range(N_CORES)], axis=0)

    if mask.any():
        out[:, :, mask] = initial_state[:, mask][:, None, :]
    return out


# revision 2
# speedup vs baseline: 1.1359x; 1.1359x over previous
"""EMA (exponential moving average) linear-recurrence kernel for TRN2, 8 cores.

y_t = w*x_t + (1-w)*y_{t-1}, inputs [B=16, T=8192, C=256] f32.

Strategy: pure data-parallel over batch (2 batches/core, no communication).
Per core, channels live on SBUF partitions (2 groups of 128) and time runs
along the free dimension, where the DVE tensor_tensor_scan instruction
computes the recurrence natively. DRAM layout is [T, C], so tiles are
transposed on-chip with the tensor engine (PE) in 128x128 blocks.

z-space trick: z_t = a*z_{t-1} + x_t with z_init = y0/w gives y_t = w*z_t,
so no pre-scale pass is needed; the w scale is folded into the transpose-back
matmul by using diag(w) instead of the identity. Channels with w ~ 0 are
fixed up on the host (y_t = y0 exactly).

Memory-bound: cost model puts this at ~99.8us/core vs a ~99.3us DMA-only
round-trip floor (33.6 MB through HBM per core). Input DMAs issue from the
SP sequencer (HWDGE) and output DMAs from GpSimd (SWDGE) to avoid
head-of-line blocking in a single issuer FIFO; batches are interleaved so
the two independent scan chains per channel group overlap.
"""

import sys

sys.path.insert(0, "/opt/trn_rl_repo")

import numpy as np

B, T, C = 16, 8192, 256
N_CORES = 8
B_LOC = B // N_CORES          # 2 batches per core
P = 128                       # SBUF partitions
G = C // P                    # 2 channel groups
TB = 512                      # timesteps per DMA block (512 KB per transfer)
NBLK = T // TB                # 16 blocks per batch
CHUNK = 512                   # timesteps per scan chunk (= 1 PSUM bank of f32)
NCHUNK = TB // CHUNK          # 1 chunk per block
SUB = CHUNK // P              # 4 PE 128x128 sub-tiles per chunk
K = TB // P                   # 4 sub-tiles per block

_compiled = None


def _build():
    import concourse.tile as tile
    from concourse import bacc, mybir
    from concourse.mybir import AluOpType

    nc = bacc.Bacc("TRN2", target_bir_lowering=False, debug=False,
                   num_devices=N_CORES)
    f32 = mybir.dt.float32

    x_ap = nc.dram_tensor("x", [B_LOC, T, C], f32, kind="ExternalInput").ap()
    abc_ap = nc.dram_tensor("abc", [P, G * CHUNK], f32, kind="ExternalInput").ap()
    wdiag_ap = nc.dram_tensor("wdiag", [P, G * P], f32, kind="ExternalInput").ap()
    ident_ap = nc.dram_tensor("ident", [P, P], f32, kind="ExternalInput").ap()
    z0c_ap = nc.dram_tensor("z0c", [P, B_LOC * G], f32, kind="ExternalInput").ap()
    y_ap = nc.dram_tensor("y", [B_LOC, T, C], f32, kind="ExternalOutput").ap()

    with tile.TileContext(nc) as tc:
        with (
            tc.tile_pool(name="const", bufs=1) as cpool,
            tc.tile_pool(name="xin", bufs=8) as xpool,
            tc.tile_pool(name="z", bufs=10) as zpool,
            tc.tile_pool(name="yout", bufs=8) as ypool,
            tc.tile_pool(name="xt", bufs=4, space="PSUM") as xtpool,
            tc.tile_pool(name="yt", bufs=4, space="PSUM") as ytpool,
        ):
            abc_t = cpool.tile([P, G * CHUNK], f32)
            nc.gpsimd.dma_start(abc_t[:], abc_ap[:])
            wdiag_t = cpool.tile([P, G * P], f32)
            nc.gpsimd.dma_start(wdiag_t[:], wdiag_ap[:])
            ident_t = cpool.tile([P, P], f32)
            nc.gpsimd.dma_start(ident_t[:], ident_ap[:])
            z0c_t = cpool.tile([P, B_LOC * G], f32)
            nc.gpsimd.dma_start(z0c_t[:], z0c_ap[:])

            zprev = {(b, g): z0c_t[:, b * G + g:b * G + g + 1]
                     for b in range(B_LOC) for g in range(G)}
            for blk in range(NBLK):
                for b in range(B_LOC):
                    t0 = blk * TB
                    xin = xpool.tile([P, K, C], f32, tag="xin")
                    src = x_ap[b, t0:t0 + TB, :].rearrange(
                        "(k p) c -> p k c", p=P)
                    nc.sync.dma_start(xin[:], src)

                    yout = ypool.tile([P, K, C], f32, tag="yout")
                    for q in range(NCHUNK):
                        for g in range(G):
                            xt = xtpool.tile([P, CHUNK], f32, tag="xt")
                            for s in range(SUB):
                                k = q * SUB + s
                                nc.tensor.transpose(
                                    xt[:, s * P:(s + 1) * P],
                                    xin[:, k, g * P:(g + 1) * P],
                                    ident_t[:],
                                )
                            z = zpool.tile([P, CHUNK], f32, tag="z")
                            nc.vector.tensor_tensor_scan(
                                z[:],
                                abc_t[:, g * CHUNK:(g + 1) * CHUNK],
                                xt[:],
                                initial=zprev[(b, g)],
                                op0=AluOpType.mult,
                                op1=AluOpType.add,
                            )
                            zprev[(b, g)] = z[:, CHUNK - 1:CHUNK]
                            yt = ytpool.tile([P, CHUNK], f32, tag="yt")
                            for s in range(SUB):
                                nc.tensor.matmul(
                                    yt[:, s * P:(s + 1) * P],
                                    z[:, s * P:(s + 1) * P],
                                    wdiag_t[:, g * P:(g + 1) * P],
                                    start=True,
                                    stop=True,
                                )
                            nc.scalar.copy(
                                yout[:, q * SUB:(q + 1) * SUB, g * P:(g + 1) * P],
                                yt[:].rearrange("p (s c) -> p s c", s=SUB),
                            )
                    dst = y_ap[b, t0:t0 + TB, :].rearrange(
                        "(k p) c -> p k c", p=P)
                    nc.gpsimd.dma_start(dst, yout[:])

    nc.compile()
    return nc


def _get_compiled():
    global _compiled
    if _compiled is None:
        _compiled = _build()
    return _compiled


def kernel(inputs, initial_state, smooth):
    from concourse.bass_utils import run_bass_kernel_spmd

    inputs = np.ascontiguousarray(inputs, dtype=np.float32)
    initial_state = np.ascontiguousarray(initial_state, dtype=np.float32)
    smooth = np.ascontiguousarray(smooth, dtype=np.float32)

    w = np.clip(smooth, 0.0, 1.0)
    a = 1.0 - w
    mask = w < 1e-30
    w_safe = np.where(mask, 1.0, w)
    z0 = initial_state / w_safe                      # [B, C]
    z0 = np.where(mask[None, :], 0.0, z0)

    # a broadcast along time, per channel group: abc[p, g*CHUNK + j] = a[g*128+p]
    abc = np.empty((P, G * CHUNK), dtype=np.float32)
    for g in range(G):
        abc[:, g * CHUNK:(g + 1) * CHUNK] = a[g * P:(g + 1) * P][:, None]
    # diag(w) per group (masked channels scale to 0; fixed up below)
    wdiag = np.zeros((P, G * P), dtype=np.float32)
    for g in range(G):
        wg = np.where(mask[g * P:(g + 1) * P], 0.0, w[g * P:(g + 1) * P])
        wdiag[:, g * P:(g + 1) * P][np.arange(P), np.arange(P)] = wg
    ident = np.eye(P, dtype=np.float32)

    nc = _get_compiled()
    in_maps = []
    for c in range(N_CORES):
        bs = slice(c * B_LOC, (c + 1) * B_LOC)
        z0c = np.empty((P, B_LOC * G), dtype=np.float32)
        for b in range(B_LOC):
            for g in range(G):
                z0c[:, b * G + g] = z0[c * B_LOC + b, g * P:(g + 1) * P]
        in_maps.append({
            "x": inputs[bs],
            "abc": abc,
            "wdiag": wdiag,
            "ident": ident,
            "z0c": z0c,
        })

    res = run_bass_kernel_spmd(nc, in_maps, list(range(N_CORES)))
    out = np.concatenate([res.results[c]["y"] for c in range(N_CORES)], axis=0)

    if mask.any():
        out[:, :, mask] = initial_state[:, mask][:, None, :]
    return out


# revision 3
# speedup vs baseline: 1.1723x; 1.0321x over previous
"""EMA (exponential moving average) linear-recurrence kernel for TRN2, 8 cores.

y_t = w*x_t + (1-w)*y_{t-1}, inputs [B=16, T=8192, C=256] f32.

Strategy: pure data-parallel over batch (2 batches/core, no communication).
Per core, channels live on SBUF partitions (2 groups of 128) and time runs
along the free dimension, where the DVE tensor_tensor_scan instruction
computes the recurrence natively. DRAM layout is [T, C], so tiles are
transposed on-chip with the tensor engine (PE) in 128x128 blocks.

z-space trick: z_t = a*z_{t-1} + x_t with z_init = y0/w gives y_t = w*z_t,
so no pre-scale pass is needed; the w scale is folded into the transpose-back
matmul by using diag(w) instead of the identity. Channels with w ~ 0 are
fixed up on the host (y_t = y0 exactly).

Memory-bound: cost model puts this at ~99.8us/core vs a ~99.3us DMA-only
round-trip floor (33.6 MB through HBM per core). Input DMAs issue from the
SP sequencer (HWDGE) and output DMAs from GpSimd (SWDGE) to avoid
head-of-line blocking in a single issuer FIFO; batches are interleaved so
the two independent scan chains per channel group overlap.
"""

import sys

sys.path.insert(0, "/opt/trn_rl_repo")

import numpy as np

B, T, C = 16, 8192, 256
N_CORES = 8
B_LOC = B // N_CORES          # 2 batches per core
P = 128                       # SBUF partitions
G = C // P                    # 2 channel groups
TB = 512                      # timesteps per DMA block (512 KB per transfer)
NBLK = T // TB                # 16 blocks per batch
CHUNK = 512                   # timesteps per scan chunk (= 1 PSUM bank of f32)
NCHUNK = TB // CHUNK          # 1 chunk per block
SUB = CHUNK // P              # 4 PE 128x128 sub-tiles per chunk
K = TB // P                   # 4 sub-tiles per block

_compiled = None


def _build():
    import concourse.tile as tile
    from concourse import bacc, mybir
    from concourse.mybir import AluOpType

    nc = bacc.Bacc("TRN2", target_bir_lowering=False, debug=False,
                   num_devices=N_CORES)
    f32 = mybir.dt.float32

    x_ap = nc.dram_tensor("x", [B_LOC, T, C], f32, kind="ExternalInput").ap()
    abc_ap = nc.dram_tensor("abc", [P, G * CHUNK], f32, kind="ExternalInput").ap()
    wdiag_ap = nc.dram_tensor("wdiag", [P, G * P], f32, kind="ExternalInput").ap()
    ident_ap = nc.dram_tensor("ident", [P, P], f32, kind="ExternalInput").ap()
    z0c_ap = nc.dram_tensor("z0c", [P, B_LOC * G], f32, kind="ExternalInput").ap()
    y_ap = nc.dram_tensor("y", [B_LOC, T, C], f32, kind="ExternalOutput").ap()

    with tile.TileContext(nc) as tc:
        with (
            tc.tile_pool(name="const", bufs=1) as cpool,
            tc.tile_pool(name="xin", bufs=8) as xpool,
            tc.tile_pool(name="z", bufs=10) as zpool,
            tc.tile_pool(name="yout", bufs=8) as ypool,
            tc.tile_pool(name="xt", bufs=4, space="PSUM") as xtpool,
            tc.tile_pool(name="yt", bufs=4, space="PSUM") as ytpool,
        ):
            abc_t = cpool.tile([P, G * CHUNK], f32)
            nc.sync.dma_start(abc_t[:], abc_ap[:])
            wdiag_t = cpool.tile([P, G * P], f32)
            nc.sync.dma_start(wdiag_t[:], wdiag_ap[:])
            ident_t = cpool.tile([P, P], f32)
            nc.sync.dma_start(ident_t[:], ident_ap[:])
            z0c_t = cpool.tile([P, B_LOC * G], f32)
            nc.sync.dma_start(z0c_t[:], z0c_ap[:])

            zprev = {(b, g): z0c_t[:, b * G + g:b * G + g + 1]
                     for b in range(B_LOC) for g in range(G)}
            for blk in range(NBLK):
                for b in range(B_LOC):
                    t0 = blk * TB
                    xin = xpool.tile([P, K, C], f32, tag="xin")
                    src = x_ap[b, t0:t0 + TB, :].rearrange(
                        "(k p) c -> p k c", p=P)
                    nc.sync.dma_start(xin[:], src)

                    yout = ypool.tile([P, K, C], f32, tag="yout")
                    for q in range(NCHUNK):
                        for g in range(G):
                            xt = xtpool.tile([P, CHUNK], f32, tag="xt")
                            for s in range(SUB):
                                k = q * SUB + s
                                nc.tensor.transpose(
                                    xt[:, s * P:(s + 1) * P],
                                    xin[:, k, g * P:(g + 1) * P],
                                    ident_t[:],
                                )
                            z = zpool.tile([P, CHUNK], f32, tag="z")
                            nc.vector.tensor_tensor_scan(
                                z[:],
                                abc_t[:, g * CHUNK:(g + 1) * CHUNK],
                                xt[:],
                                initial=zprev[(b, g)],
                                op0=AluOpType.mult,
                                op1=AluOpType.add,
                            )
                            zprev[(b, g)] = z[:, CHUNK - 1:CHUNK]
                            yt = ytpool.tile([P, CHUNK], f32, tag="yt")
                            for s in range(SUB):
                                nc.tensor.matmul(
                                    yt[:, s * P:(s + 1) * P],
                                    z[:, s * P:(s + 1) * P],
                                    wdiag_t[:, g * P:(g + 1) * P],
                                    start=True,
                                    stop=True,
                                )
                            nc.scalar.copy(
                                yout[:, q * SUB:(q + 1) * SUB, g * P:(g + 1) * P],
                                yt[:].rearrange("p (s c) -> p s c", s=SUB),
                            )
                    dst = y_ap[b, t0:t0 + TB, :].rearrange(
                        "(k p) c -> p k c", p=P)
                    nc.gpsimd.dma_start(dst, yout[:])

    nc.compile()
    return nc


def _get_compiled():
    global _compiled
    if _compiled is None:
        _compiled = _build()
    return _compiled


def kernel(inputs, initial_state, smooth):
    from concourse.bass_utils import run_bass_kernel_spmd

    inputs = np.ascontiguousarray(inputs, dtype=np.float32)
    initial_state = np.ascontiguousarray(initial_state, dtype=np.float32)
    smooth = np.ascontiguousarray(smooth, dtype=np.float32)

    w = np.clip(smooth, 0.0, 1.0)
    a = 1.0 - w
    mask = w < 1e-30
    w_safe = np.where(mask, 1.0, w)
    z0 = initial_state / w_safe                      # [B, C]
    z0 = np.where(mask[None, :], 0.0, z0)

    # a broadcast along time, per channel group: abc[p, g*CHUNK + j] = a[g*128+p]
    abc = np.empty((P, G * CHUNK), dtype=np.float32)
    for g in range(G):
        abc[:, g * CHUNK:(g + 1) * CHUNK] = a[g * P:(g + 1) * P][:, None]
    # diag(w) per group (masked channels scale to 0; fixed up below)
    wdiag = np.zeros((P, G * P), dtype=np.float32)
    for g in range(G):
        wg = np.where(mask[g * P:(g + 1) * P], 0.0, w[g * P:(g + 1) * P])
        wdiag[:, g * P:(g + 1) * P][np.arange(P), np.arange(P)] = wg
    ident = np.eye(P, dtype=np.float32)

    nc = _get_compiled()
    in_maps = []
    for c in range(N_CORES):
        bs = slice(c * B_LOC, (c + 1) * B_LOC)
        z0c = np.empty((P, B_LOC * G), dtype=np.float32)
        for b in range(B_LOC):
            for g in range(G):
                z0c[:, b * G + g] = z0[c * B_LOC + b, g * P:(g + 1) * P]
        in_maps.append({
            "x": inputs[bs],
            "abc": abc,
            "wdiag": wdiag,
            "ident": ident,
            "z0c": z0c,
        })

    res = run_bass_kernel_spmd(nc, in_maps, list(range(N_CORES)))
    out = np.concatenate([res.results[c]["y"] for c in range(N_CORES)], axis=0)

    if mask.any():
        out[:, :, mask] = initial_state[:, mask][:, None, :]
    return out


# revision 5
# speedup vs baseline: 1.1774x; 1.0043x over previous
"""EMA (exponential moving average) linear-recurrence kernel for TRN2, 8 cores.

y_t = w*x_t + (1-w)*y_{t-1}, inputs [B=16, T=8192, C=256] f32.

Strategy: pure data-parallel over batch (2 batches/core, no communication).
Per core, channels live on SBUF partitions (2 groups of 128) and time runs
along the free dimension, where the DVE tensor_tensor_scan instruction
computes the recurrence natively. DRAM layout is [T, C], so tiles are
transposed on-chip with the tensor engine (PE) in 128x128 blocks.

z-space trick: z_t = a*z_{t-1} + x_t with z_init = y0/w gives y_t = w*z_t,
so no pre-scale pass is needed; the w scale is folded into the transpose-back
matmul by using diag(w) instead of the identity. Channels with w ~ 0 are
fixed up on the host (y_t = y0 exactly).

Memory-bound: cost model puts this at ~99.4us/core vs a ~99.3us DMA-only
round-trip floor (33.6 MB through HBM per core). Input DMAs issue from the
SP sequencer and output DMAs from ACT (both HWDGE) to avoid head-of-line
blocking in a single issuer FIFO; batches are interleaved so the two
independent scan chains per channel group overlap.
"""

import sys

sys.path.insert(0, "/opt/trn_rl_repo")

import numpy as np

B, T, C = 16, 8192, 256
N_CORES = 8
B_LOC = B // N_CORES          # 2 batches per core
P = 128                       # SBUF partitions
G = C // P                    # 2 channel groups
TB = 512                      # timesteps per DMA block (512 KB per transfer)
NBLK = T // TB                # 16 blocks per batch
CHUNK = 512                   # timesteps per scan chunk (= 1 PSUM bank of f32)
NCHUNK = TB // CHUNK          # 1 chunk per block
SUB = CHUNK // P              # 4 PE 128x128 sub-tiles per chunk
K = TB // P                   # 4 sub-tiles per block

_compiled = None


def _build():
    import concourse.tile as tile
    from concourse import bacc, mybir
    from concourse.mybir import AluOpType

    nc = bacc.Bacc("TRN2", target_bir_lowering=False, debug=False,
                   num_devices=N_CORES)
    f32 = mybir.dt.float32

    x_ap = nc.dram_tensor("x", [B_LOC, T, C], f32, kind="ExternalInput").ap()
    abc_ap = nc.dram_tensor("abc", [P, G * CHUNK], f32, kind="ExternalInput").ap()
    wdiag_ap = nc.dram_tensor("wdiag", [P, G * P], f32, kind="ExternalInput").ap()
    ident_ap = nc.dram_tensor("ident", [P, P], f32, kind="ExternalInput").ap()
    z0c_ap = nc.dram_tensor("z0c", [P, B_LOC * G], f32, kind="ExternalInput").ap()
    y_ap = nc.dram_tensor("y", [B_LOC, T, C], f32, kind="ExternalOutput").ap()

    with tile.TileContext(nc) as tc:
        with (
            tc.tile_pool(name="const", bufs=1) as cpool,
            tc.tile_pool(name="xin", bufs=8) as xpool,
            tc.tile_pool(name="z", bufs=10) as zpool,
            tc.tile_pool(name="yout", bufs=8) as ypool,
            tc.tile_pool(name="xt", bufs=4, space="PSUM") as xtpool,
            tc.tile_pool(name="yt", bufs=4, space="PSUM") as ytpool,
        ):
            abc_t = cpool.tile([P, G * CHUNK], f32)
            nc.sync.dma_start(abc_t[:], abc_ap[:])
            wdiag_t = cpool.tile([P, G * P], f32)
            nc.sync.dma_start(wdiag_t[:], wdiag_ap[:])
            ident_t = cpool.tile([P, P], f32)
            nc.sync.dma_start(ident_t[:], ident_ap[:])
            z0c_t = cpool.tile([P, B_LOC * G], f32)
            nc.sync.dma_start(z0c_t[:], z0c_ap[:])

            zprev = {(b, g): z0c_t[:, b * G + g:b * G + g + 1]
                     for b in range(B_LOC) for g in range(G)}
            for blk in range(NBLK):
                for b in range(B_LOC):
                    t0 = blk * TB
                    xin = xpool.tile([P, K, C], f32, tag="xin")
                    src = x_ap[b, t0:t0 + TB, :].rearrange(
                        "(k p) c -> p k c", p=P)
                    nc.sync.dma_start(xin[:], src)

                    yout = ypool.tile([P, K, C], f32, tag="yout")
                    for q in range(NCHUNK):
                        for g in range(G):
                            xt = xtpool.tile([P, CHUNK], f32, tag="xt")
                            for s in range(SUB):
                                k = q * SUB + s
                                nc.tensor.transpose(
                                    xt[:, s * P:(s + 1) * P],
                                    xin[:, k, g * P:(g + 1) * P],
                                    ident_t[:],
                                )
                            z = zpool.tile([P, CHUNK], f32, tag="z")
                            nc.vector.tensor_tensor_scan(
                                z[:],
                                abc_t[:, g * CHUNK:(g + 1) * CHUNK],
                                xt[:],
                                initial=zprev[(b, g)],
                                op0=AluOpType.mult,
                                op1=AluOpType.add,
                            )
                            zprev[(b, g)] = z[:, CHUNK - 1:CHUNK]
                            yt = ytpool.tile([P, CHUNK], f32, tag="yt")
                            for s in range(SUB):
                                nc.tensor.matmul(
                                    yt[:, s * P:(s + 1) * P],
                                    z[:, s * P:(s + 1) * P],
                                    wdiag_t[:, g * P:(g + 1) * P],
                                    start=True,
                                    stop=True,
                                )
                            nc.scalar.copy(
                                yout[:, q * SUB:(q + 1) * SUB, g * P:(g + 1) * P],
                                yt[:].rearrange("p (s c) -> p s c", s=SUB),
                            )
                    dst = y_ap[b, t0:t0 + TB, :].rearrange(
                        "(k p) c -> p k c", p=P)
                    nc.scalar.dma_start(dst, yout[:])

    nc.compile()
    return nc


def _get_compiled():
    global _compiled
    if _compiled is None:
        _compiled = _build()
    return _compiled


def kernel(inputs, initial_state, smooth):
    from concourse.bass_utils import run_bass_kernel_spmd

    inputs = np.ascontiguousarray(inputs, dtype=np.float32)
    initial_state = np.ascontiguousarray(initial_state, dtype=np.float32)
    smooth = np.ascontiguousarray(smooth, dtype=np.float32)

    w = np.clip(smooth, 0.0, 1.0)
    a = 1.0 - w
    mask = w < 1e-30
    w_safe = np.where(mask, 1.0, w)
    z0 = initial_state / w_safe                      # [B, C]
    z0 = np.where(mask[None, :], 0.0, z0)

    # a broadcast along time, per channel group: abc[p, g*CHUNK + j] = a[g*128+p]
    abc = np.empty((P, G * CHUNK), dtype=np.float32)
    for g in range(G):
        abc[:, g * CHUNK:(g + 1) * CHUNK] = a[g * P:(g + 1) * P][:, None]
    # diag(w) per group (masked channels scale to 0; fixed up below)
    wdiag = np.zeros((P, G * P), dtype=np.float32)
    for g in range(G):
        wg = np.where(mask[g * P:(g + 1) * P], 0.0, w[g * P:(g + 1) * P])
        wdiag[:, g * P:(g + 1) * P][np.arange(P), np.arange(P)] = wg
    ident = np.eye(P, dtype=np.float32)

    nc = _get_compiled()
    in_maps = []
    for c in range(N_CORES):
        bs = slice(c * B_LOC, (c + 1) * B_LOC)
        z0c = np.empty((P, B_LOC * G), dtype=np.float32)
        for b in range(B_LOC):
            for g in range(G):
                z0c[:, b * G + g] = z0[c * B_LOC + b, g * P:(g + 1) * P]
        in_maps.append({
            "x": inputs[bs],
            "abc": abc,
            "wdiag": wdiag,
            "ident": ident,
            "z0c": z0c,
        })

    res = run_bass_kernel_spmd(nc, in_maps, list(range(N_CORES)))
    out = np.concatenate([res.results[c]["y"] for c in range(N_CORES)], axis=0)

    if mask.any():
        out[:, :, mask] = initial_state[:, mask][:, None, :]
    return out
